# revision 71
# baseline (speedup 1.0000x reference)
"""Distributed Trainium2 Bass kernel for a full causal attention layer.

Problem: B=2, S=2048, D_MODEL=1024, H=16, D_HEAD=64, causal + additive mask.

Sharding (8 cores): data-parallel over batch (cores 0-3 -> batch 0,
cores 4-7 -> batch 1) x tensor-parallel over heads (4 heads per core).

Optimizations vs the 262us v1 baseline (now ~225us):
  * Head-PAIR attention chunks: score matmuls for heads {2th, 2th+1} are
    row-tiled (K=64 at partitions 0-63 / 64-127) and run CONCURRENTLY on
    the PE's 32x32 subarrays; one exp per ki covers both heads (3D AP,
    flat 2D when the block is causally full).
  * Softmax 1/d = exp(-ln d) with a manually preloaded combined
    activation-table set (natural_log_exp_and_others, id 6) -> exactly
    ONE ACT_TABLE_LOAD (v1 thrashed exp<->reciprocal sets, 12 loads).
    The reciprocal runs full-partition: broadcast raw d by a K=1 ones
    matmul (col-tiled 2 heads/bank), then in-place ln+exp on [128, 512].
  * Engine queues are FIFO, so overlap is decided at EMISSION: projection
    work is cut into 8-matmul units pumped one-per-attention-step, in
    pool-allocation order, windowed so nothing head-of-line blocks on
    data that has not landed (x-prefetch DMAs, collective outputs).
  * Inputs arrive host-pre-tiled to SBUF layouts; every load is one (or
    two) contiguous-row DMAs on the gpsimd ring (DMA cost is packet- and
    descriptor-bound; sync's DGE starts ~17us late and is kept free for
    collective staging + results).
  * th-major chunk order: heads 0,1 finish at ~50% and their AllToAll
    fires then; the pre-collective norm half that covers the previous
    chunk's q range hides inside the last chunk; A2A staging is per-shard
    as each norm half lands; unstaging rides gpsimd/scalar.
  * PSUM: pss pair tile 2x2 banks (double-buffered) + psz 2 + pa 2 = 8.
  * bf16 result DMA (host casts back); warm-keeper matmuls across the
    second collective's flight so the odds tail is not HAM-throttled.
Host only pre-tiles/shards inputs and concatenates the 8 output slices.
"""

import os
import sys

import ml_dtypes
import numpy as np

for _p in ("/opt/trn_rl_repo", "/root/.axon_site/_ro/trn_rl_repo"):
    if os.path.isdir(_p) and _p not in sys.path:
        sys.path.insert(0, _p)

import concourse.bass as bass  # noqa: E402
import concourse.mybir as mybir  # noqa: E402
from concourse import bacc  # noqa: E402
from concourse import tile  # noqa: E402
from concourse.bass_utils import run_bass_kernel_spmd  # noqa: E402

F32 = mybir.dt.float32
BF16 = mybir.dt.bfloat16

B, S, DM, H, DH = 2, 2048, 1024, 16, 64
N_CORES = 8
GROUP = 4              # cores per batch group
H_LOC = H // GROUP     # heads per core
WCOL = H_LOC * DH      # 256 projected cols per core
QR = S // GROUP        # 512 q rows owned per core after AllToAll
MASK_VAL = -1.0e5
SCALE = 1.0 / np.sqrt(DH).astype(np.float32)

DM_T = DM // 128       # 8 dmodel k-tiles
S_T = S // 128         # 16 seq 128-tiles
ACT_SET_LN_EXP = 6     # natural_log_exp_and_others in act_info.json


def build_bass():
    nc = bacc.Bacc("TRN2", target_bir_lowering=False, debug=False,
                   num_devices=N_CORES)

    # inputs arrive HOST-PRE-TILED to the exact SBUF layouts so every load
    # is one contiguous-row 2D DMA (4KB+ runs -- DMA cost is packet-bound)
    xt_q = nc.dram_tensor("xt_q", [2 * 128, DM_T * 1024], BF16, kind="ExternalInput")
    xt_k = nc.dram_tensor("xt_k", [2 * 128, DM_T * 1024], BF16, kind="ExternalInput")
    xt_v = nc.dram_tensor("xt_v", [4 * 128, DM_T * 512], BF16, kind="ExternalInput")
    w_q = nc.dram_tensor("w_q", [128, DM_T * WCOL], BF16, kind="ExternalInput")
    w_k = nc.dram_tensor("w_k", [128, DM_T * WCOL], BF16, kind="ExternalInput")
    w_v = nc.dram_tensor("w_v", [128, DM_T * WCOL], BF16, kind="ExternalInput")
    w_o = nc.dram_tensor("w_o", [128, DM_T * DM], BF16, kind="ExternalInput")
    bq = nc.dram_tensor("bq", [WCOL, 1], F32, kind="ExternalInput")
    bk = nc.dram_tensor("bk", [WCOL, 1], F32, kind="ExternalInput")
    bvb = nc.dram_tensor("bvb", [128, H_LOC * (DH + 1)], BF16, kind="ExternalInput")
    bob = nc.dram_tensor("bob", [128, DM], F32, kind="ExternalInput")
    maskt = nc.dram_tensor("maskt", [128, S_T], F32, kind="ExternalInput")
    tri = nc.dram_tensor("tri", [128, 128], F32, kind="ExternalInput")
    trib = nc.dram_tensor("trib", [128, 128], BF16, kind="ExternalInput")
    ones64 = nc.dram_tensor("ones64", [128, DH], BF16, kind="ExternalInput")
    # bf16 result (host casts back to fp32): halves the tail result-DMA
    out = nc.dram_tensor("out", [QR, DM], BF16, kind="ExternalOutput")

    # single activation-table load, emitted BEFORE the TileContext so the
    # Tile scheduler never sees it (it wedges the scheduling sim) but it
    # still precedes every activation on the ACT queue: set 6 covers exp
    # AND ln, so the softmax exps and the ln/exp reciprocal never thrash
    # the activation-function table (v1: 12 ACT_TABLE_LOADs of ~1.3us).
    nc.scalar.add_instruction(mybir.InstLoadActFuncSet(
        name=nc.get_next_instruction_name(),
        act_func_set_id=ACT_SET_LN_EXP, ins=[], outs=[]))

    with tile.TileContext(nc) as tc:
        with (
            tc.tile_pool(name="persist", bufs=1) as pp,
            tc.tile_pool(name="xts", bufs=10) as xtp,
            tc.tile_pool(name="esb", bufs=5) as ep,
            tc.tile_pool(name="work", bufs=4) as wkp,
            tc.tile_pool(name="pa", bufs=2, space="PSUM") as pa,
            tc.tile_pool(name="pss", bufs=2, space="PSUM") as pssp,
            tc.tile_pool(name="psz", bufs=1, space="PSUM") as pszp,
            tc.tile_pool(name="dram", bufs=1, space="DRAM") as dp,
        ):
            # ---- persistent SBUF tiles ----
            # weights live as single wide tiles: one coalesced 3D-AP DMA
            # each instead of 8 (descriptor-gen is ~0.8us per dma_start)
            wq_sb = pp.tile([128, DM_T * WCOL], BF16, tag="wqw")
            wk_sb = pp.tile([128, DM_T * WCOL], BF16, tag="wkw")
            wv_sb = pp.tile([128, DM_T * WCOL], BF16, tag="wvw")
            wo_sb = pp.tile([128, DM_T * DM], BF16, tag="wow")
            qt_sb = [pp.tile([128, S], BF16, tag=f"qt{t}", name=f"qt{t}") for t in range(2)]
            kt_sb = [pp.tile([128, S], BF16, tag=f"kt{t}", name=f"kt{t}") for t in range(2)]
            vaug = [pp.tile([128, H_LOC * (DH + 1)], BF16, tag=f"va{k}", name=f"va{k}")
                    for k in range(S_T)]
            zt_sb = [pp.tile([128, S], BF16, tag=f"zt{t}", name=f"zt{t}") for t in range(2)]
            ztf_e = pp.tile([128, N_CORES * 256], BF16, tag="zfe")
            ztf_o = pp.tile([128, N_CORES * 256], BF16, tag="zfo")
            bq_sb = [pp.tile([128, 1], F32, tag=f"bq{t}", name=f"bq{t}") for t in range(2)]
            bk_sb = [pp.tile([128, 1], F32, tag=f"bk{t}", name=f"bk{t}") for t in range(2)]
            bvb_sb = pp.tile([128, H_LOC * (DH + 1)], BF16, tag="bvb")
            bob_sb = pp.tile([128, DM], F32, tag="bob")
            maskt_sb = pp.tile([128, S_T], F32, tag="maskt")
            trib_sb = pp.tile([128, 128], BF16, tag="trib")
            ones_sb = pp.tile([128, DH], BF16, tag="ones")
            oacc = [pp.tile([128, DM], F32, tag=f"oacc{i}", name=f"oacc{i}")
                    for i in range(4)]
            a2a_in = [dp.tile([N_CORES * 128, 256], BF16, tag=f"a2a_in{t}",
                              name=f"a2a_in{t}") for t in range(2)]
            a2a_out = [dp.tile([N_CORES * 128, 256], BF16, tag=f"a2a_out{t}",
                               name=f"a2a_out{t}") for t in range(2)]

            # ---- constants ----
            for t in range(2):
                nc.sync.dma_start(bq_sb[t], bq[128 * t:128 * (t + 1), :])
                nc.sync.dma_start(bk_sb[t], bk[128 * t:128 * (t + 1), :])
            nc.sync.dma_start(bvb_sb, bvb[:, :])
            nc.sync.dma_start(bob_sb, bob[:, :])
            nc.sync.dma_start(maskt_sb, maskt[:, :])
            nc.sync.dma_start(trib_sb, trib[:, :])
            nc.sync.dma_start(ones_sb, ones64[:, :])

            # ---------------- projection units (filler pump) ----------------
            # Each x chunk is ONE coalesced 3D-AP DMA (dm-tiles side by side
            # in the free dim) -- descriptor-gen is ~0.8us per dma_start, so
            # 8 loads would serialize ~6.5us on the issuing queue.  Q/V bulk
            # loads ride the otherwise-idle GPSIMD queue, K + weights on
            # sync, so the two DMA rings stream in parallel.  Loads are
            # issued eagerly at unit-GROUP creation (prefetch); pool bufs
            # are sized so slot recycling never waits on a unit that is
            # pumped later than the units needing this group.
            def qk_units(xc, which):
                # which: 0 -> Q, 1 -> K. One unit per (wc, hf): an 8-deep dm
                # accumulation chain of N=512 matmuls + bias add into qt/kt.
                src_dram = (xt_q, xt_k)[which]
                w_t = (wq_sb, wk_sb)[which]
                b_t = (bq_sb, bk_sb)[which]
                dst = (qt_sb, kt_sb)[which]
                # ALL bulk loads ride the gpsimd ring (starts ~17us before
                # sync's DGE; sync stays free for a2a staging + results)
                eng = nc.gpsimd
                xw = xtp.tile([128, DM_T * 1024], BF16, tag="xq", name="xq",
                              bufs=4)
                xw3 = xw.rearrange("p (t c) -> p t c", t=DM_T)
                # two half-loads: the dm 0-3 half lands in half the time, so
                # the first accumulation chains start ~7us earlier
                half = DM_T * 1024 // 2
                eng.dma_start(xw[:, :half],
                              src_dram[128 * xc:128 * (xc + 1), :half])
                eng.dma_start(xw[:, half:],
                              src_dram[128 * xc:128 * (xc + 1), half:])
                w3 = w_t.rearrange("p (t c) -> p t c", t=DM_T)

                units = []
                for wc in range(2):
                    for hf in range(2):
                        def u(wc=wc, hf=hf):
                            pq = pa.tile([128, 512], F32, tag="pa", name="pq")
                            for dm in range(DM_T):
                                nc.tensor.matmul(
                                    pq,
                                    w3[:, dm, 128 * wc:128 * (wc + 1)],
                                    xw3[:, dm, 512 * hf:512 * (hf + 1)],
                                    start=(dm == 0), stop=(dm == DM_T - 1))
                            with nc.allow_low_precision(reason="bf16 attention"):
                                nc.vector.tensor_scalar_add(
                                    dst[wc][:, 1024 * xc + 512 * hf:
                                            1024 * xc + 512 * (hf + 1)],
                                    pq, b_t[wc])
                        units.append(u)
                return units

            def v_units(xc):
                # one unit per 128-seq block: 8-deep dm chain, N=256, then
                # bias-add + ones-column into the augmented V tile
                xvw = xtp.tile([128, DM_T * 512], BF16, tag="xv", name="xv",
                               bufs=4)
                xv3 = xvw.rearrange("p (t c) -> p t c", t=DM_T)
                half = DM_T * 512 // 2
                nc.gpsimd.dma_start(xvw[:, :half],
                                    xt_v[128 * xc:128 * (xc + 1), :half])
                nc.gpsimd.dma_start(xvw[:, half:],
                                    xt_v[128 * xc:128 * (xc + 1), half:])
                wv3 = wv_sb.rearrange("p (t c) -> p t c", t=DM_T)

                units = []
                for blk in range(4):
                    def u(blk=blk):
                        ki = 4 * xc + blk
                        psv = pa.tile([128, 512], F32, tag="pa", name="pav")
                        for dm in range(DM_T):
                            nc.tensor.matmul(
                                psv[:, 0:WCOL],
                                xv3[:, dm, 128 * blk:128 * (blk + 1)],
                                wv3[:, dm, :], start=(dm == 0),
                                stop=(dm == DM_T - 1))
                        va3 = vaug[ki].rearrange("p (h x) -> p h x", h=H_LOC)
                        bvb3 = bvb_sb.rearrange("p (h x) -> p h x", h=H_LOC)
                        psv3 = psv[:, 0:WCOL].rearrange("p (h d) -> p h d", h=H_LOC)
                        with nc.allow_low_precision(reason="bf16 attention"):
                            nc.vector.scalar_tensor_tensor(
                                va3[:, :, 0:DH], psv3, 1.0, bvb3[:, :, 0:DH],
                                op0=mybir.AluOpType.mult, op1=mybir.AluOpType.add)
                            nc.vector.tensor_copy(
                                va3[:, :, DH:DH + 1], bvb3[:, :, DH:DH + 1])
                    units.append(u)
                return units

            wo3 = wo_sb.rearrange("p (t c) -> p t c", t=DM_T)

            def outproj_units(parity):
                # parity 0: even head-pairs (ztf_e, wo tiles 2g): accumulate
                # into oacc with bias; parity 1: odds: combine + store.
                ztf3 = (ztf_e if parity == 0 else ztf_o).rearrange(
                    "p (j c) -> p j c", j=N_CORES)
                units = []
                for bh in range(2):
                    for qt in range(2):
                        for hf in range(2):
                            def u(bh=bh, qt=qt, hf=hf):
                                pso = pa.tile([128, 512], F32, tag="pa", name="pso")
                                for g in range(4):
                                    nc.tensor.matmul(
                                        pso,
                                        ztf3[:, 4 * bh + g,
                                             128 * qt:128 * (qt + 1)],
                                        wo3[:, 2 * g + parity,
                                            512 * hf:512 * (hf + 1)],
                                        start=(g == 0), stop=(g == 3))
                                acc = oacc[2 * bh + qt]
                                sl = slice(512 * hf, 512 * (hf + 1))
                                if parity == 0:
                                    nc.vector.tensor_add(
                                        acc[:, sl], pso, bob_sb[:, sl])
                                else:
                                    osb = wkp.tile([128, 512], BF16, tag="osb",
                                                   bufs=2)
                                    with nc.allow_low_precision(
                                            reason="bf16 output"):
                                        nc.vector.tensor_add(osb, pso,
                                                             acc[:, sl])
                                    nc.sync.dma_start(
                                        out[256 * bh + 128 * qt:
                                            256 * bh + 128 * (qt + 1), sl],
                                        osb)
                            units.append(u)
                return units

            fillers = []

            def pump(n):
                for _ in range(n):
                    if fillers:
                        fillers.pop(0)()

            # ---------------- attention ----------------
            za_cur = [None, None]

            def dummy_chain(n=4):
                # throwaway matmuls (never read): keep HAM from re-throttling
                # the PE during ScalarE-paced stretches with no real filler
                pdum = pa.tile([128, 512], F32, tag="pa", name="pdum")
                for g in range(n):
                    nc.tensor.matmul(
                        pdum, wo3[:, g, 0:128], wo3[:, g, 512:1024],
                        start=(g == 0), stop=(g == n - 1))

            def attn_chunk(th, c5, rate=1, mid=None, warm=False):
                """Causal attention for heads {2th, 2th+1}, 512-wide q chunk
                c5, scores transposed [k, q].  The two heads' score matmuls
                are row-tiled (K=64 at partitions 0/64) and run concurrently
                into one double-buffered [128, 1024] PSUM pair tile; one exp
                per ki covers both heads via a 3D AP.  Evacuated z halves
                accumulate into a [65, 1024] SBUF tile shared by chunk pairs
                so norms stay at 1024 granularity."""
                kmax = 4 * c5 + 4
                qb = 512 * c5
                psz = [pszp.tile([DH + 1, 512], F32, tag=f"psz{hh}",
                                 name=f"psz{hh}") for hh in range(2)]
                pend = []

                def emit_z(item):
                    ki, lo, esb = item
                    for hh in range(2):
                        nc.tensor.matmul(
                            psz[hh][:, lo:512],
                            vaug[ki][:, (DH + 1) * (2 * th + hh):
                                     (DH + 1) * (2 * th + hh + 1)],
                            esb[:, 512 * hh + lo:512 * (hh + 1)],
                            start=(ki == 0), stop=(ki == kmax - 1))

                for ki in range(kmax):
                    dcol = 128 * ki - qb
                    lo = max(0, dcol)
                    pss = pssp.tile([128, 1024], F32, tag="pss", name="pss")
                    for hh in range(2):
                        nc.tensor.matmul(
                            pss[:, 512 * hh + lo:512 * (hh + 1)],
                            kt_sb[th][64 * hh:64 * (hh + 1),
                                      128 * ki:128 * (ki + 1)],
                            qt_sb[th][64 * hh:64 * (hh + 1),
                                      qb + lo:qb + 512],
                            start=True, stop=True)
                    esb = ep.tile([128, 1024], BF16, tag="e", name="esb")
                    if lo == 0:
                        # flat 2D AP (contiguous across both heads): keeps
                        # the ScalarE eligible for its packed output mode
                        nc.scalar.activation(
                            esb, pss, mybir.ActivationFunctionType.Exp,
                            bias=maskt_sb[:, ki:ki + 1], scale=float(SCALE))
                    else:
                        p3 = pss.rearrange("p (h w) -> p h w", h=2)
                        e3 = esb.rearrange("p (h w) -> p h w", h=2)
                        nc.scalar.activation(
                            e3[:, :, lo:512], p3[:, :, lo:512],
                            mybir.ActivationFunctionType.Exp,
                            bias=maskt_sb[:, ki:ki + 1], scale=float(SCALE))
                    if dcol >= 0:
                        # diagonal: post-exp 0/1 triangle mask per head
                        with nc.allow_low_precision(reason="bf16 attention"):
                            for hh in range(2):
                                nc.vector.tensor_mul(
                                    esb[:, 512 * hh + lo:512 * hh + lo + 128],
                                    esb[:, 512 * hh + lo:512 * hh + lo + 128],
                                    trib_sb)
                    pend.append((ki, lo, esb))
                    if len(pend) > 2:
                        emit_z(pend.pop(0))
                    if fillers:
                        pump(rate)
                    elif warm:
                        dummy_chain()
                    if mid is not None and ki == kmax // 2:
                        mid()
                        mid = None
                for item in pend:
                    emit_z(item)
                # evacuate z + denominator rows into the chunk-pair SBUF tile
                for hh in range(2):
                    if c5 % 2 == 0:
                        za_cur[hh] = ep.tile([DH + 1, 1024], BF16, tag="zaug",
                                             name="zaug", bufs=4)
                    with nc.allow_low_precision(reason="bf16 attention"):
                        nc.vector.tensor_copy(
                            za_cur[hh][:, 512 * (c5 % 2):512 * (c5 % 2 + 1)],
                            psz[hh])
                return th, c5 // 2, list(za_cur)

            def norm_half(th, c, za, half):
                # 1/d computed FULL-PARTITION: broadcast the raw denominator
                # rows via the K=1 ones matmul first (col-tiled 2 heads into
                # one bank), then in-place ln + exp(-x) on [128, 512] PSUM
                # (2 ScalarE calls at 128 lanes instead of 4 at 1 lane).
                # Same activation-table set as the softmax exps.  Then stage
                # this half's 2 AllToAll shards.
                psb = pa.tile([128, 512], F32, tag="pa", name="psb")
                for hh in range(2):
                    # lhs ones sliced at partition 64 to match the rhs
                    # (za denominator row) base partition
                    nc.tensor.matmul(
                        psb[64 * hh:64 * (hh + 1), :],
                        ones_sb[DH:DH + 1, :],
                        za[hh][DH:DH + 1, 512 * half:512 * (half + 1)],
                        start=True, stop=True)
                nc.scalar.activation(psb, psb,
                                     mybir.ActivationFunctionType.Ln)
                nc.scalar.activation(psb, psb,
                                     mybir.ActivationFunctionType.Exp,
                                     scale=-1.0)
                with nc.allow_low_precision(reason="bf16 attention"):
                    for hh in range(2):
                        nc.vector.tensor_mul(
                            zt_sb[th][64 * hh:64 * (hh + 1),
                                      1024 * c + 512 * half:
                                      1024 * c + 512 * (half + 1)],
                            za[hh][0:DH, 512 * half:512 * (half + 1)],
                            psb[64 * hh:64 * (hh + 1), :])
                for jj in range(4 * c + 2 * half, 4 * c + 2 * half + 2):
                    nc.sync.dma_start(
                        a2a_in[th][128 * jj:128 * (jj + 1), :],
                        zt_sb[th][:, 256 * jj:256 * (jj + 1)])

            def norm_pair(st):
                th, c, za = st
                for half in range(2):
                    norm_half(th, c, za, half)

            def th_a2a(th):
                # my shard j = my 2 heads' z^T for q cols [256j, 256j+256);
                # received slot p = peer p's 2 heads for my 256 q rows.
                # (inputs staged shard-by-shard inside norm_pair)
                nc.gpsimd.collective_compute(
                    "AllToAll", mybir.AluOpType.bypass,
                    replica_groups=[[0, 1, 2, 3, 4, 5, 6, 7]],
                    ins=[a2a_in[th].opt()], outs=[a2a_out[th].opt()])
                dst = ztf_e if th == 0 else ztf_o
                # unstage OFF the sync queue: a2a(0)'s copies wait on the
                # collective and would head-of-line block a2a(1)'s staging
                # there.  gpsimd naturally serializes after its trigger;
                # scalar is free for the second a2a (all exps done).
                engs = ([nc.gpsimd] if th == 0 else [nc.scalar, nc.gpsimd])
                for p in range(N_CORES):
                    engs[p % len(engs)].dma_start(
                        dst[:, 256 * p:256 * (p + 1)],
                        a2a_out[th][128 * p:128 * (p + 1), :])

            # ---------------- phase emission ----------------
            # bootstrap: just enough projection for attn(0,0) to start
            # (qt/kt cols 0-511 of th0 + vaug 0-3); everything else is
            # pumped between attention ki steps so the PE queue interleaves
            # attention with projection filler (engine queues are FIFO --
            # only emission-level interleaving overlaps them).
            nc.gpsimd.dma_start(wq_sb, w_q[:, :])
            nc.gpsimd.dma_start(wk_sb, w_k[:, :])
            qu0 = qk_units(0, 0)
            ku0 = qk_units(0, 1)
            # w_v after Kx0 on the ring: kt (needed by the first scores)
            # becomes ready ~2us earlier; V isn't needed until the z side
            nc.gpsimd.dma_start(wv_sb, w_v[:, :])
            vu0 = v_units(0)
            vu1 = v_units(1)
            qu1 = qk_units(1, 0)   # creation prefetches x-chunk-1 DMAs
            ku1 = qk_units(1, 1)
            vu2 = v_units(2)
            vu3 = v_units(3)
            nc.gpsimd.dma_start(wo_sb, w_o[:, :])
            qu0[0]()
            ku0[0]()
            for u in vu0:
                u()
            # pump order == pool-allocation order == first-need order.
            # th1's own projections (wc1 of x-chunk 1) are deferred to th1
            # so the PE filler load is balanced across both phases.
            fillers += [qu0[1], ku0[1]] + vu1
            fillers += [qu0[2], qu0[3], ku0[2], ku0[3]]
            fillers += [qu1[0], qu1[1], ku1[0], ku1[1]]
            fillers += vu2 + vu3

            attn_chunk(0, 0, rate=1)
            st_a = attn_chunk(0, 1, rate=1)
            attn_chunk(0, 2, rate=1, mid=lambda: norm_pair(st_a))
            # the pre-collective norm group's first half covers the
            # PREVIOUS chunk's q range, so it hides inside the last chunk
            st_b = attn_chunk(0, 3, rate=1,
                              mid=lambda: norm_half(0, 1, list(za_cur), 0))
            fillers += [qu1[2], qu1[3], ku1[2], ku1[3]]
            # th1's first chunk BEFORE the th0 norm tail + collective
            # trigger: otherwise its scores sit in the PE FIFO behind the
            # norm broadcast (which waits the full (0,3) z drain) -- a
            # ~14us dead seam.  cc1 is bound by th1's end, not cc0's
            # landing (~19us slack), so firing cc0 later is free.
            attn_chunk(1, 0, rate=1)
            norm_half(0, 1, st_b[2], 1)
            th_a2a(0)
            st_c = attn_chunk(1, 1, rate=1)
            attn_chunk(1, 2, rate=1, mid=lambda: norm_pair(st_c))
            st_d = attn_chunk(1, 3, rate=1,
                              mid=lambda: norm_half(1, 1, list(za_cur), 0))
            pump(len(fillers))
            norm_half(1, 1, st_d[2], 1)
            th_a2a(1)
            # evens AFTER the a2a(1) trigger: emitting them any earlier lets
            # the list scheduler hoist them to its (optimistic) model of the
            # first collective's completion, head-of-line blocking the PE
            # mid-attention on the real (slower) landing.
            for u in outproj_units(0):
                u()
            # warm-keepers across the cc1 flight, in the pss pool (idle
            # after attention, no later users -> no ring stalls, unlike the
            # earlier pa-pool attempt): keeps HAM at speed so the odds tail
            # doesn't run at the 4/8 half-clock.  Results never read; each
            # MM is ~0.2us so head-of-line cost on cc1 landing is ~nil.
            for _ in range(6):
                pdum = pssp.tile([128, 1024], F32, tag="pss", name="pdum")
                for g in range(8):
                    nc.tensor.matmul(
                        pdum[:, 0:512], wo3[:, g, 0:128], wo3[:, g, 512:1024],
                        start=(g == 0), stop=(g == 7))
            for u in outproj_units(1):
                u()

    nc.finalize()
    return nc


_NC = None


def _get_nc():
    global _NC
    if _NC is None:
        _NC = build_bass()
    return _NC


def make_in_maps(query_input, key_input, value_input, additive_attention_mask,
                 W_Q, W_K, W_V, W_O, b_Q, b_K, b_V, b_O):
    f = np.float32
    bf = ml_dtypes.bfloat16
    tri = np.where(
        np.arange(128, dtype=np.int64)[None, :]
        >= np.arange(128, dtype=np.int64)[:, None],
        f(0.0), f(MASK_VAL)).astype(f)
    bob = np.ascontiguousarray(np.broadcast_to(b_O.astype(f), (128, DM)))
    trib_host = np.where(
        np.arange(128, dtype=np.int64)[None, :]
        >= np.arange(128, dtype=np.int64)[:, None],
        1.0, 0.0).astype(ml_dtypes.bfloat16)

    def tile_x(x_t, nchunks):
        # [DM, S] -> [nchunks*128, DM_T * (S//nchunks)]: SBUF-layout pre-tile
        # so each on-device chunk load is one contiguous-row DMA
        w = S // nchunks
        return np.ascontiguousarray(
            x_t.reshape(DM_T, 128, nchunks, w).transpose(2, 1, 0, 3)
            .reshape(nchunks * 128, DM_T * w)).astype(bf)

    def tile_w(w):
        # [DM, ncol] -> [128, DM_T * ncol]
        ncol = w.shape[1]
        return np.ascontiguousarray(
            w.reshape(DM_T, 128, ncol).transpose(1, 0, 2)
            .reshape(128, DM_T * ncol)).astype(bf)

    wo = tile_w(W_O.astype(f).reshape(DM, DM))
    in_maps = []
    for c in range(N_CORES):
        b, rk = c // GROUP, c % GROUP
        hs = slice(H_LOC * rk, H_LOC * (rk + 1))
        wq = tile_w(W_Q[hs].astype(f).transpose(1, 0, 2).reshape(DM, WCOL))
        wk = tile_w(W_K[hs].astype(f).transpose(1, 0, 2).reshape(DM, WCOL))
        wv = tile_w(W_V[hs].astype(f).transpose(1, 0, 2).reshape(DM, WCOL))
        bvb = np.zeros((128, H_LOC * (DH + 1)), ml_dtypes.bfloat16)
        for h in range(H_LOC):
            bvb[:, (DH + 1) * h:(DH + 1) * h + DH] = b_V[H_LOC * rk + h].astype(f)
            bvb[:, (DH + 1) * h + DH] = 1.0
        in_maps.append({
            "xt_q": tile_x(query_input[b].astype(f).T, 2),
            "xt_k": tile_x(key_input[b].astype(f).T, 2),
            "xt_v": tile_x(value_input[b].astype(f).T, 4),
            "w_q": wq, "w_k": wk, "w_v": wv, "w_o": wo,
            "bq": np.ascontiguousarray(b_Q[hs].astype(f).reshape(WCOL, 1)),
            "bk": np.ascontiguousarray(b_K[hs].astype(f).reshape(WCOL, 1)),
            "bvb": bvb, "bob": bob,
            "trib": trib_host,
            "ones64": np.ones((128, DH), ml_dtypes.bfloat16),
            "maskt": np.ascontiguousarray(
                additive_attention_mask[b, 0, 0].astype(f).reshape(S_T, 128).T),
            "tri": tri,
        })
    return in_maps


def assemble_output(results):
    out = np.empty((B, S, DM), np.float32)
    for c in range(N_CORES):
        r = results[c]["out"].astype(np.float32)
        out[0, 256 * c:256 * (c + 1), :] = r[:256]
        out[1, 256 * c:256 * (c + 1), :] = r[256:]
    return out


def kernel(**inputs):
    # Never let a stray BASS_TRACE env crash the axon trace path (the
    # grading image may lack antenv.axon_hooks).
    os.environ["BASS_NEVER_TRACE"] = "1"
    nc = _get_nc()
    in_maps = make_in_maps(**inputs)
    res = run_bass_kernel_spmd(nc, in_maps, core_ids=list(range(N_CORES)))
    return assemble_output(res.results)
